# revision 18
# baseline (speedup 1.0000x reference)
"""Causal self-attention (B=2, T=2048, C=1024, H=16, D=64) on 8 trn2 NeuronCores.

Sharding: data-parallel over batch (2) x tensor-parallel over heads (16 -> 4
per core). Core c handles batch c//4 and head-quad c%4 (feature slice of 256).
Each core computes q/k/v projections for its 4 heads, causal attention, and a
partial output projection against its 256-column slice of Wo. The host sums
the 4 partials per batch (the TP all-reduce) and adds bo + Wo @ bv (the value
bias contributes exactly Wo @ bv per token since attention rows sum to 1).

v2 design notes:
- All operands are bf16 (host-cast); matmuls accumulate fp32 in PSUM. This
  keeps every PE matmul at 1 cycle/row (fp32r pays 4x below 256-wide moving)
  and halves HBM + SBUF traffic.
- x and all weights are transposed by the DMA xbar engine on load
  (dma_start_transpose, 2-byte dtype) straight into the [contraction-major]
  layouts the PE needs: zero PE transposes, zero PSUM->SBUF transpose copies.
- The k bias is dropped entirely: softmax over keys is invariant to the
  per-query constant q . bk. The q bias is folded into the q^T PSUM->SBUF
  copy-out as a per-partition tensor_scalar_add (bias laid out [128,2] by the
  host), so there are no bias matmuls on the PE.
- Scores are computed transposed (S^T[k, q]) so the scalar engine's exp
  writes P^T directly in the layout the P@V matmul consumes; softmax runs
  without max-subtraction (logits bounded) and the denominator comes from an
  appended ones-column in the V stationary operand. The softmax reciprocal
  uses the single-instruction approx-fast DVE op (~51 ULP, plenty here).
- The output partial is stored bf16 and upconverted on the host, halving
  store traffic. Stores go out through the GPSIMD SWDGE path and the next
  chunk's x-transpose is queued ahead of them: the HWDGE queue serializes
  on xbar-mode transitions (transpose vs copy), so keeping stores off that
  queue and transposes early is worth ~18us/iter on hardware.
- The emission order interleaves the projection "prep" work for q-chunk qc+1
  (and the output projection for qc-1) into the attention t-loops of q-chunk
  qc via a deferred work queue, so the PE always has independent work while
  the scalar engine grinds through exp.
"""

import numpy as np

B = 2
T = 2048
C = 1024
NH = 16
D = 64
HEADS_PER_CORE = 4
FSLICE = HEADS_PER_CORE * D  # 256 features per core
SCALE = 0.125  # 1/sqrt(64)
N_CORES = 8

TOKB = T // 128  # 16 token blocks
KCH = C // 128  # 8 contraction chunks
QCH = T // 512  # 4 q chunks

import os
EXACT_RECIP = os.environ.get("EXACT_RECIP", "0") == "1"
SCORES_F32R = os.environ.get("SCORES_F32R", "0") == "1"
SPLIT_X0 = os.environ.get("SPLIT_X0", "1") == "1"
VMOVE = os.environ.get("VMOVE", "1") == "1"
HINT_PE = os.environ.get("HINT_PE", "1") == "1"
STORE_POOL = os.environ.get("STORE_POOL", "1") == "1"
MASK_TT = os.environ.get("MASK_TT", "1") == "1"


def _build_nc(repeat=1):
    from collections import deque
    from contextlib import ExitStack, nullcontext

    import concourse.bacc as bacc
    import concourse.mybir as mb
    import concourse.tile as tile

    F32 = mb.dt.float32
    BF16 = mb.dt.bfloat16

    nc = bacc.Bacc()
    x_d = nc.dram_tensor("x", [T, C], BF16, kind="ExternalInput")
    wq_d = nc.dram_tensor("wq", [FSLICE, C], BF16, kind="ExternalInput")
    wk_d = nc.dram_tensor("wk", [FSLICE, C], BF16, kind="ExternalInput")
    wv_d = nc.dram_tensor("wv", [FSLICE, C], BF16, kind="ExternalInput")
    wo_d = nc.dram_tensor("wo", [C, FSLICE], BF16, kind="ExternalInput")
    bqt_d = nc.dram_tensor("bqt", [128, 2], F32, kind="ExternalInput")
    out_d = nc.dram_tensor("out", [T, C], BF16, kind="ExternalOutput")

    with tile.TileContext(nc) as tc, ExitStack() as top:
        # ---- persistent SBUF ----
        perm = top.enter_context(tc.tile_pool(name="perm", bufs=1))
        wqT = perm.tile([128, KCH, FSLICE], BF16)  # [c, kc, feat]
        wkT = perm.tile([128, KCH, FSLICE], BF16)
        wvT = perm.tile([128, KCH, FSLICE], BF16)
        woT = perm.tile([128, 2, C], BF16)  # [feat, fc, out]
        bqT = perm.tile([128, 2], F32)  # q bias, feature-major [f%128, f//128]
        QKDT = mb.dt.float32r if SCORES_F32R else BF16
        qT = [
            [perm.tile([128, 512], QKDT, name=f"qT{p}_{qc}") for qc in range(QCH)]
            for p in range(2)
        ]
        kT = [
            [perm.tile([128, 512], QKDT, name=f"kT{p}_{qc}") for qc in range(QCH)]
            for p in range(2)
        ]
        # D+2 stride keeps per-head slices 4B-aligned; col D is the ones
        # column for the softmax denominator, col D+1 is padding.
        v_sb = [
            perm.tile([128, HEADS_PER_CORE, D + 2], BF16, name=f"v{tb}")
            for tb in range(TOKB)
        ]
        oT = [
            [perm.tile([128, 512], BF16, name=f"oT{p}_{qc}") for qc in range(QCH)]
            for p in range(2)
        ]
        for tb in range(TOKB):
            nc.vector.memset(
                v_sb[tb][:, :, D : D + 1].rearrange("p a c -> p (a c)"), 1.0
            )
        # lower-triangle keep-mask for the diagonal band: mask[k, j] = j >= k
        mask_tri = perm.tile([128, 128], BF16, name="mask_tri")
        nc.vector.memset(mask_tri, 1.0)
        nc.gpsimd.affine_select(
            out=mask_tri, in_=mask_tri, compare_op=mb.AluOpType.is_ge,
            fill=0.0, base=0, channel_multiplier=-1, pattern=[[1, 128]],
        )

        xtp = top.enter_context(tc.tile_pool(name="xtp", bufs=3))
        scps = top.enter_context(tc.tile_pool(name="scps", bufs=2, space="PSUM"))
        pvps = top.enter_context(tc.tile_pool(name="pvps", bufs=2, space="PSUM"))
        wkps = top.enter_context(tc.tile_pool(name="wkps", bufs=2, space="PSUM"))
        ptp = top.enter_context(tc.tile_pool(name="ptp", bufs=6))
        rcp = top.enter_context(tc.tile_pool(name="rcp", bufs=8))
        outp = top.enter_context(tc.tile_pool(name="outp", bufs=6))

        xTq = {}  # qc -> rotating [c, kc, tok-chunk] tile

        def x_item(qc, split=False):
            xTq[qc] = xtp.tile([128, KCH, 512], BF16, name="xTq")
            if split:
                # per-kc transposes so the first qk matmul can start as soon
                # as its contraction chunk lands (startup path only)
                for kc in range(KCH):
                    nc.sync.dma_start_transpose(
                        out=xTq[qc][:, kc, :],
                        in_=x_d[qc * 512 : (qc + 1) * 512, kc * 128 : (kc + 1) * 128],
                    )
            else:
                nc.sync.dma_start_transpose(
                    out=xTq[qc], in_=x_d[qc * 512 : (qc + 1) * 512, :]
                )

        def load_items():
            return [
                lambda: nc.sync.dma_start_transpose(out=wqT, in_=wq_d[:, :]),
                lambda: x_item(0, split=SPLIT_X0),
                lambda: nc.sync.dma_start_transpose(out=wkT, in_=wk_d[:, :]),
                lambda: nc.sync.dma_start_transpose(out=wvT, in_=wv_d[:, :]),
                lambda: nc.gpsimd.dma_start(out=bqT, in_=bqt_d[:, :]),
                lambda: nc.sync.dma_start_transpose(out=woT, in_=wo_d[:, :]),
            ]

        def qk_items(qc, ps=(0, 1)):
            """q/k projection for one q-chunk; one psum reused across items."""
            items = []
            for p in ps:
                for wT, dstT, is_q in ((wqT, qT, True), (wkT, kT, False)):
                    box = {}
                    for kc in range(KCH):
                        def step(wT=wT, p=p, kc=kc, box=box):
                            if kc == 0:
                                box[0] = wkps.tile([128, 512], F32, name="wk_ps")
                            nc.tensor.matmul(
                                box[0],
                                wT[:, kc, p * 128 : (p + 1) * 128],
                                xTq[qc][:, kc, :],
                                start=(kc == 0),
                                stop=(kc == KCH - 1),
                            )
                        items.append(step)
                    def fin(dstT=dstT, p=p, box=box, is_q=is_q):
                        if is_q:
                            nc.vector.tensor_scalar_add(
                                dstT[p][qc], box[0], bqT[:, p : p + 1]
                            )
                        else:
                            nc.vector.tensor_copy(dstT[p][qc], box[0])
                    items.append(fin)
            return items

        def v_item(tb):
            vps = wkps.tile([128, FSLICE], F32, name="wk_ps")
            for kc in range(KCH):
                nc.tensor.matmul(
                    vps,
                    xTq[tb // 4][:, kc, (tb % 4) * 128 : (tb % 4 + 1) * 128],
                    wvT[:, kc, :],
                    start=(kc == 0),
                    stop=(kc == KCH - 1),
                )
            nc.vector.tensor_copy(
                v_sb[tb][:, :, 0:D],
                vps.rearrange("p (h d) -> p h d", h=HEADS_PER_CORE),
            )

        def proj_item(qc, tb4):
            tb = qc * 4 + tb4
            pj = scps.tile([128, 1024], F32, name="pj", tag="sc")
            for oc in range(2):
                for p in range(2):
                    nc.tensor.matmul(
                        pj[:, oc * 512 : (oc + 1) * 512],
                        oT[p][qc][:, tb4 * 128 : (tb4 + 1) * 128],
                        woT[:, p, oc * 512 : (oc + 1) * 512],
                        start=(p == 0),
                        stop=(p == 1),
                    )
            ostage = outp.tile([128, C], BF16, name="ostage")
            nc.vector.tensor_copy(ostage, pj)
            # SWDGE stores keep the HWDGE queue free of xbar-mode transitions
            eng = nc.gpsimd if STORE_POOL else nc.sync
            eng.dma_start(out=out_d[tb * 128 : (tb + 1) * 128, :], in_=ostage)

        def qk_items_with_v(qc):
            items = qk_items(qc)
            if not VMOVE:
                items += [
                    (lambda tb=tb: v_item(tb)) for tb in range(qc * 4, qc * 4 + 4)
                ]
            return items

        def attention_sweep(extra_front=None):
            for qc in range(QCH):
                ntb = 4 * qc + 4
                items = []
                # next chunk's x-transpose goes FIRST: the xbar-mode
                # transition workaround serializes it against regular DMAs,
                # so it must beat this sweep's output stores into the queue
                if qc + 1 < QCH:
                    items += [lambda q=qc + 1: x_item(q)]
                if qc == 0 and extra_front:
                    items += extra_front
                if qc > 0:
                    if VMOVE:
                        # own v first: consumed by this sweep's diagonal PVs
                        items += [
                            (lambda tb=tb: v_item(tb))
                            for tb in range(qc * 4, qc * 4 + 4)
                        ]
                    items += [
                        (lambda tb4=tb4, q=qc - 1: proj_item(q, tb4))
                        for tb4 in range(4)
                    ]
                if qc + 1 < QCH:
                    items += qk_items_with_v(qc + 1)
                queue = deque(items)
                points = {"left": 2 * ntb}  # injection points in this qc

                def inject():
                    per_pop = max(1, -(-len(queue) // max(1, points["left"])))
                    points["left"] -= 1
                    n = 0
                    while queue and n < per_pop:
                        queue.popleft()()
                        n += 1

                for p in range(2):
                    pv0 = pvps.tile([65, 512], F32, name="pv", tag="pv")
                    pv1 = pvps.tile([65, 512], F32, name="pv", tag="pv")
                    pvs = (pv0, pv1)
                    pts = {}

                    def emit_scores(tb, p=p, qc=qc, pts=pts):
                        e = max(0, tb * 128 - qc * 512)
                        sc = scps.tile([128, 1024], F32, name="sc", tag="sc")
                        for h2 in range(2):
                            nc.tensor.matmul(
                                sc[:, h2 * 512 + e : (h2 + 1) * 512],
                                kT[p][tb // 4][
                                    h2 * 64 : (h2 + 1) * 64,
                                    (tb % 4) * 128 : (tb % 4 + 1) * 128,
                                ],
                                qT[p][qc][h2 * 64 : (h2 + 1) * 64, e:],
                                start=True,
                                stop=True,
                            )
                        pt = ptp.tile([128, 2, 512], BF16, name="pt")
                        nc.scalar.activation(
                            pt[:, :, e:],
                            sc.rearrange("p (h q) -> p h q", h=2)[:, :, e:],
                            mb.ActivationFunctionType.Exp,
                            scale=SCALE,
                        )
                        if tb >= 4 * qc:
                            # zero the sub-diagonal triangle in the 128-wide
                            # band [e, e+128); cols < e are never read later.
                            # DVE mask-mult keeps this off the POOL queue
                            # (now busy with SWDGE store descriptor-gen).
                            if MASK_TT:
                                for h2 in range(2):
                                    nc.vector.tensor_tensor(
                                        out=pt[:, h2, e : e + 128],
                                        in0=pt[:, h2, e : e + 128],
                                        in1=mask_tri,
                                        op=mb.AluOpType.mult,
                                    )
                            else:
                                nc.gpsimd.affine_select(
                                    out=pt[:, :, e : e + 128],
                                    in_=pt[:, :, e : e + 128],
                                    compare_op=mb.AluOpType.is_ge,
                                    fill=0.0,
                                    base=0,
                                    channel_multiplier=-1,
                                    pattern=[[0, 2], [1, 128]],
                                )
                        pts[tb] = pt

                    def emit_pv(tb, p=p, qc=qc, ntb=ntb, pvs=pvs, pts=pts):
                        e = max(0, tb * 128 - qc * 512)
                        pt = pts.pop(tb)
                        for h2 in range(2):
                            nc.tensor.matmul(
                                pvs[h2][:, e:],
                                v_sb[tb][:, 2 * p + h2, 0 : D + 1],
                                pt[:, h2, e:],
                                start=(tb == 0),
                                stop=(tb == ntb - 1),
                            )

                    # pipeline: PV trails scores by two t-blocks; queue
                    # work injected before each dependent PV
                    emit_scores(0)
                    if ntb > 1:
                        emit_scores(1)
                    for tb in range(2, ntb):
                        emit_scores(tb)
                        inject()
                        emit_pv(tb - 2)
                    inject()
                    emit_pv(ntb - 2)
                    emit_pv(ntb - 1)
                    inject()
                    for h2 in range(2):
                        recip = rcp.tile([1, 512], F32, name="recip")
                        if EXACT_RECIP:
                            with nc.allow_low_precision(reason="softmax denom"):
                                nc.vector.reciprocal(recip, pvs[h2][64:65, :])
                        else:
                            # approx_fast misreads PSUM sources on HW; stage
                            # the denominator row through SBUF first
                            den = rcp.tile([1, 512], F32, name="den")
                            nc.vector.tensor_copy(den, pvs[h2][64:65, :])
                            nc.vector.reciprocal_approx_fast(recip, den)
                        bc_sb = rcp.tile([64, 512], F32, name="bc_sb")
                        nc.gpsimd.partition_broadcast(bc_sb, recip)
                        nc.vector.tensor_tensor(
                            out=oT[p][qc][h2 * 64 : (h2 + 1) * 64, :],
                            in0=pvs[h2][0:64, :],
                            in1=bc_sb,
                            op=mb.AluOpType.mult,
                        )
                while queue:
                    queue.popleft()()
            for tb4 in range(4):
                proj_item(QCH - 1, tb4)

        def body():
            for it in load_items():
                it()
            for it in qk_items(0, ps=(0,)) + [
                (lambda tb=tb: v_item(tb)) for tb in range(4)
            ]:
                it()
            attention_sweep(extra_front=qk_items(0, ps=(1,)))

        # PE body far exceeds one 256-instr IRAM block; hint the back-edge
        # target so the loop branch I$-hits (~300ns) instead of stalling
        # ~3-4us on an IRAM refetch each iteration.
        hints = (mb.EngineType.PE,) if HINT_PE else ()
        loop_ctx = (
            tc.For_i(0, repeat, 1, hint_engines=hints)
            if repeat > 1
            else nullcontext()
        )
        with loop_ctx:
            body()

    nc.compile()
    return nc


_NC_CACHE = {}


def _get_nc(repeat=1):
    if repeat not in _NC_CACHE:
        _NC_CACHE[repeat] = _build_nc(repeat)
    return _NC_CACHE[repeat]


def make_in_maps(x, Wq, bq, Wk, bk, Wv, bv, Wo, bo):
    import ml_dtypes

    BF = ml_dtypes.bfloat16
    x = np.asarray(x, dtype=np.float32)
    in_maps = []
    for c in range(N_CORES):
        b, p4 = divmod(c, 4)
        fs = slice(p4 * FSLICE, (p4 + 1) * FSLICE)
        bqt = np.asarray(bq, np.float32)[fs].reshape(2, 128).T  # [128, 2]
        in_maps.append(
            {
                "x": np.ascontiguousarray(x[b]).astype(BF),
                "wq": np.ascontiguousarray(np.asarray(Wq)[fs, :]).astype(BF),
                "wk": np.ascontiguousarray(np.asarray(Wk)[fs, :]).astype(BF),
                "wv": np.ascontiguousarray(np.asarray(Wv)[fs, :]).astype(BF),
                "wo": np.ascontiguousarray(np.asarray(Wo)[:, fs]).astype(BF),
                "bqt": np.ascontiguousarray(bqt),
            }
        )
    return in_maps


def combine_outputs(outs, Wo, bv, bo):
    """outs: list of 8 [T, C] bf16 partials. Host-side TP all-reduce + biases."""
    const = np.asarray(bo, dtype=np.float32) + np.asarray(
        Wo, dtype=np.float32
    ) @ np.asarray(bv, dtype=np.float32)
    full = (
        np.stack([np.asarray(o).astype(np.float32) for o in outs])
        .reshape(B, 4, T, C)
        .sum(axis=1, dtype=np.float32)
    )
    return (full + const[None, None, :]).astype(np.float32)


def kernel(x, Wq, bq, Wk, bk, Wv, bv, Wo, bo):
    from concourse.bass_utils import run_bass_kernel_spmd

    nc = _get_nc()
    in_maps = make_in_maps(x, Wq, bq, Wk, bk, Wv, bv, Wo, bo)
    res = run_bass_kernel_spmd(nc, in_maps, core_ids=list(range(N_CORES)))
    outs = [res.results[c]["out"] for c in range(N_CORES)]
    return combine_outputs(outs, Wo, bv, bo)
